# revision 15
# baseline (speedup 1.0000x reference)
"""Trainium2 Bass kernel for nn_CholecFixScore (pairwise-IoU mask scoring).

Math (per sample n):
    Gp (P=16, HW) and Gt (T=8, HW) are binary {0,1} masks.
    inters[p,t] = sum_hw Gp[p]*Gt[t];  sp[p] = sum Gp[p];  st[t] = sum Gt[t]
    iou = inters / (sp+st-inters)                  (union==0 never occurs for
                                                    randint mask fills)
    w[p] = max_t iou[p,t]
    den[hw] = sum_p Gp[p,hw];  r = 1/max(den,1)    (den==0 pixels have Gp==0)
    score[n] = (1/HW) * sum_p w[p] * S[p],  S[p] = sum_hw Gp[p,hw]*r[hw]
which equals the reference's mean over pixels of (sum_p w[p]Gp[p,hw])/den[hw].

Sharding: pure data parallel, 2 samples per core on 8 cores.

Precision: masks are {0,1} so bf16 operands are exact and all PE sums
accumulate exactly in fp32 PSUM.  r = 1/den is shipped as ONE bf16 rhs
column: S picks up a <= 2^-9 relative error (all-positive sum), far
inside the 2e-2 gate; everything else is exact integer arithmetic.

On-chip layout: pixel index hw = part*392 + j  (part=0..127, j=0..391).
    gp_sb (128, 17*392) bf16  free = (p, j), p=16 is a ones "mask" (memset)
    gt_sb (128, 10*392) bf16  free = (u, j), u = 8 Gt | ones | r
Main pass: per sample, 392 accumulating bf16 matmuls (one per j):
    lhsT = gp_sb viewed (part, p, j)[:, :, j]  -> (128, 17), col stride 392
    rhs  = gt_sb viewed (part, j, u)[:, j, :]  -> (128, 10), col stride 392
    out  = ps_acc (17, 10) accumulated over all 392 j
(2D strided APs straight out of the natural layouts -- the BIR verifier
rejects 3D weights APs, so no j-blocking; Ldweights costs no engine time
in the hardware model and the moving cost is out-columns only, so 392
small matmuls equal 56 blocked ones on PE cycles with no shuffle pass
and no block-diagonal extraction.)
ps_acc rows 0..15 = [inters | sp | S] per mask p; row 16 (the ones mask)
= [st | HW | sum r].  One (17,16) all-e16 selector matmul broadcasts
row 16 onto partitions 0..15 for the unions computation.

Input DMAs: 6 large SWDGE cast DMAs (fp32->bf16), 1024 descriptors each
(994ns fixed overhead amortized); issue order Gp0a,Gp0b,Gp1a,Gp1b,Gt0,Gt1
so the den reduction for sample 1 finishes under the Gt transfers and the
two main passes run back-to-back keeping PE at max p-state.
"""

import numpy as np

import concourse.bass as bass
import concourse.tile as tile
from concourse import mybir
from concourse.bass_utils import run_bass_kernel_spmd

F32 = mybir.dt.float32
BF16 = mybir.dt.bfloat16
ADD = mybir.AluOpType.add
SUB = mybir.AluOpType.subtract
MULT = mybir.AluOpType.mult

N, P, T = 16, 16, 8
H, W = 224, 224
HW = H * W            # 50176
PART = 128
JW = HW // PART       # 392 columns per mask
PCOL = P + 1          # 17 lhsT columns: 16 masks + ones
U = T + 2             # 10 rhs columns: 8 Gt | ones | r
ONES_C = T * JW       # col offset of ones region in gt_sb
R_C = (T + 1) * JW    # col offset of r region in gt_sb
GONES_C = P * JW      # col offset of ones "mask" in gp_sb
NCORES = 8
SPC = N // NCORES     # samples per core = 2
INV_HW = 1.0 / HW


def _split_multi_waits(nc):
    """The pinned walrus encodes only ONE sync-wait per instruction; split
    Tile-emitted multi-wait instructions into single-wait NOPs ahead of them
    (same engine, program order => identical semantics)."""
    n = 0
    for f in nc.m.functions:
        for bb in f.blocks:
            insts = bb.instructions
            newlist = []
            changed = False
            for ins in insts:
                si = ins.sync_info
                if si is not None and si.on_wait is not None and len(si.on_wait) > 1:
                    waits = list(si.on_wait)
                    for w in waits[:-1]:
                        n += 1
                        newlist.append(
                            mybir.InstNoOp(
                                name=f"I-waitsplit-{n}",
                                engine=ins.engine,
                                ins=[],
                                outs=[],
                                sync_info=mybir.SyncInfo(on_wait=[w], on_update=[]),
                            )
                        )
                    ins.sync_info = mybir.SyncInfo(
                        on_wait=[waits[-1]], on_update=list(si.on_update or [])
                    )
                    changed = True
                newlist.append(ins)
            if changed:
                while len(insts):
                    insts.pop()
                for x in newlist:
                    insts.append(x)
    return n


def _build():
    nc = bass.Bass("TRN2", target_bir_lowering=False, debug=False)
    gp = nc.dram_tensor("gp", [SPC, P, PART, JW], F32, kind="ExternalInput")
    gt = nc.dram_tensor("gt", [SPC, T, PART, JW], F32, kind="ExternalInput")
    # ce row 16 is all-ones: lhsT (17,16) broadcasting acc row 16 (st) onto
    # partitions 0..15
    ce = nc.dram_tensor("ce", [PART, 16], F32, kind="ExternalInput")
    y = nc.dram_tensor("y", [1, SPC], F32, kind="ExternalOutput")

    with tile.TileContext(nc) as tc:
        with (
            tc.tile_pool(name="big", bufs=2) as big,
            tc.tile_pool(name="scratch", bufs=1) as scratch,
            tc.tile_pool(name="small", bufs=2) as small,
            tc.tile_pool(name="singles", bufs=1) as singles,
            tc.tile_pool(name="psmain", bufs=2, space="PSUM") as psmain,
            tc.tile_pool(name="psaux", bufs=1, space="PSUM") as psaux,
        ):
            e_sb = singles.tile([PART, 16], F32)
            ones16c = singles.tile([16, 1], F32)
            out_sb = singles.tile([1, SPC], F32)

            gps, gts = [], []
            for s in range(SPC):
                gps.append(big.tile([PART, PCOL * JW], BF16, tag="gp", name=f"gp_sb{s}"))
                gts.append(big.tile([PART, U * JW], BF16, tag="gt", name=f"gt_sb{s}"))

            # constants / ones regions (DVE + sync HWDGE; Pool stays free for
            # the input SWDGE stream)
            with tc.high_priority():
                nc.sync.dma_start(out=e_sb[:, :], in_=ce[:, :])
                for s in range(SPC):
                    nc.vector.memset(gps[s][:, GONES_C : GONES_C + JW], 1.0)
                    nc.vector.memset(gts[s][:, ONES_C : ONES_C + JW], 1.0)
                nc.vector.memset(ones16c[:, :], 1.0)

            # ---- input DMAs: 6 big SWDGE cast DMAs (1024 descriptors each)
            def dma_gp(s, lo, hi):
                src = gp[s, lo:hi, :, :].rearrange("p part j -> part p j")
                dst = gps[s][:].rearrange("part (p j) -> part p j", j=JW)[:, lo:hi, :]
                nc.gpsimd.dma_start(out=dst, in_=src)

            def dma_gt(s):
                src = gt[s, :, :, :].rearrange("t part j -> part t j")
                dst = gts[s][:].rearrange("part (u j) -> part u j", j=JW)[:, 0:T, :]
                nc.gpsimd.dma_start(out=dst, in_=src)

            dma_gp(0, 0, 8)
            dma_gp(0, 8, 16)
            dma_gp(1, 0, 8)
            dma_gp(1, 8, 16)
            dma_gt(0)
            dma_gt(1)

            # ---- den = sum_p Gp[p] (DVE wide log-tree, one 8-mask half at a
            # time so each half folds as soon as its DMA lands) ----
            def den_half(s, h):
                g = gps[s]
                base = h * 8 * JW
                x = scratch.tile([PART, 4 * JW], BF16, tag=f"x{s}", name=f"x{s}_{h}")
                y4 = scratch.tile([PART, 2 * JW], BF16, tag=f"y{s}", name=f"y{s}_{h}")
                dh = scratch.tile([PART, JW], BF16, tag=f"dh{s}{h}", name=f"dh{s}_{h}")
                nc.vector.tensor_tensor(
                    x[:], g[:, base : base + 4 * JW],
                    g[:, base + 4 * JW : base + 8 * JW], ADD,
                )
                nc.vector.tensor_tensor(y4[:], x[:, 0 : 2 * JW], x[:, 2 * JW : 4 * JW], ADD)
                nc.vector.tensor_tensor(dh[:], y4[:, 0:JW], y4[:, JW : 2 * JW], ADD)
                return dh

            def r_chain(s, dA, dB):
                den = scratch.tile([PART, JW], BF16, tag=f"den{s}", name=f"den{s}")
                nc.vector.tensor_tensor(den[:], dA[:], dB[:], ADD)
                nc.vector.tensor_scalar_max(out=den[:], in0=den[:], scalar1=1.0)
                # reciprocal writes the bf16 rhs column directly (<=2^-9 rel
                # error on S through the all-positive matmul sum; gate is 2e-2)
                with nc.allow_low_precision(reason="bf16 r column, 2e-2 gate"):
                    nc.vector.reciprocal(out=gts[s][:, R_C : R_C + JW], in_=den[:])

            # ---- main pass: 392 accumulating (17x10) matmuls per sample
            ps_accs = {}

            def main_pass(s):
                gp_v = gps[s][:].rearrange("part (p j) -> part p j", j=JW)
                gt_v = gts[s][:].rearrange("part (u j) -> part j u", j=JW)
                ps_acc = psmain.tile([PCOL, U], F32, tag="main", name=f"ps_acc{s}")
                ps_accs[s] = ps_acc
                for j in range(JW):
                    nc.tensor.matmul(
                        ps_acc[:, :],
                        gp_v[:, :, j],
                        gt_v[:, j, :],
                        start=(j == 0),
                        stop=(j == JW - 1),
                    )

            accs = {}

            def acc_copy(s):
                acc = small.tile([PCOL, U], F32, tag="accsb", name=f"acc{s}")
                nc.vector.tensor_copy(acc[:, :], ps_accs[s][:, :])
                accs[s] = acc

            def finish_pre(s):
                acc = accs[s]
                # broadcast st (acc row 16) onto partitions 0..15
                ps_st16 = psaux.tile([16, T], F32, tag="st16", name=f"ps_st16{s}")
                nc.tensor.matmul(ps_st16[:, :], e_sb[0:PCOL, :], acc[0:PCOL, 0:T])
                unions = small.tile([16, T], F32, tag="un", name=f"unions{s}")
                nc.vector.scalar_tensor_tensor(
                    out=unions[:, :],
                    in0=ps_st16[:, :],
                    scalar=acc[0:16, T : T + 1],
                    in1=acc[0:16, 0:T],
                    op0=ADD,
                    op1=SUB,
                )
                nc.vector.reciprocal(out=unions[:, :], in_=unions[:, :])
                iou = small.tile([16, T], F32, tag="iou", name=f"iou{s}")
                wmax = small.tile([16, 1], F32, tag="wm", name=f"wmax{s}")
                nc.vector.tensor_tensor(iou[:, :], acc[0:16, 0:T], unions[:, :], MULT)
                nc.vector.tensor_reduce(
                    out=wmax[:, :], in_=iou[:, :],
                    axis=mybir.AxisListType.X, op=mybir.AluOpType.max,
                )
                ws = small.tile([16, 1], F32, tag="ws", name=f"ws{s}")
                nc.vector.tensor_tensor(ws[:, :], acc[0:16, T + 1 : T + 2], wmax[:, :], MULT)
                return ws

            def finish_post(s, ws):
                ps_score = psaux.tile([1, 1], F32, tag="sc", name=f"ps_score{s}")
                nc.tensor.matmul(ps_score[:, :], ones16c[:, :], ws[:, :])
                nc.vector.tensor_scalar_mul(
                    out=out_sb[0:1, s : s + 1], in0=ps_score[:, :], scalar1=INV_HW
                )

            # program order per engine == issue order.  PE: the two main
            # passes back-to-back (keeps the p-state ramp), then the tiny
            # broadcast/score matmuls.  DVE: den/r chains, then acc copies,
            # then both epilogues.
            dA0 = den_half(0, 0)
            dB0 = den_half(0, 1)
            r_chain(0, dA0, dB0)
            dA1 = den_half(1, 0)
            dB1 = den_half(1, 1)
            r_chain(1, dA1, dB1)
            main_pass(0)
            acc_copy(0)
            main_pass(1)
            acc_copy(1)
            ws0 = finish_pre(0)
            finish_post(0, ws0)
            ws1 = finish_pre(1)
            finish_post(1, ws1)

            nc.sync.dma_start(out=y[:, :], in_=out_sb[:, :])

    _split_multi_waits(nc)
    return nc


_NC = None


def _get_nc():
    global _NC
    if _NC is None:
        _NC = _build()
    return _NC


def kernel(groups_pred: np.ndarray, groups_true: np.ndarray) -> np.ndarray:
    assert groups_pred.shape == (N, P, H, W)
    assert groups_true.shape == (N, T, H, W)
    gp = np.ascontiguousarray(groups_pred, dtype=np.float32).reshape(
        NCORES, SPC, P, PART, JW
    )
    gt = np.ascontiguousarray(groups_true, dtype=np.float32).reshape(
        NCORES, SPC, T, PART, JW
    )
    ce = np.zeros((PART, 16), dtype=np.float32)
    ce[16, :] = 1.0
    in_maps = [{"gp": gp[c], "gt": gt[c], "ce": ce} for c in range(NCORES)]
    res = run_bass_kernel_spmd(_get_nc(), in_maps, core_ids=list(range(NCORES)))
    out = np.empty((N,), dtype=np.float32)
    for c in range(NCORES):
        out[c * SPC : (c + 1) * SPC] = res.results[c]["y"][0]
    return out
